# revision 24
# baseline (speedup 1.0000x reference)
"""DirectVoxGO Raw2Alpha + Alphas2Weights (segmented scan) on 8 Trainium2 cores.

Computes, for n_pts sample points sorted by ray_id:
    sp      = softplus(density + shift)
    log(1-alpha) = -interval * sp
    weights = alpha * T   (T = exclusive per-ray cumprod of (1-alpha))
    alphainv_last[r] = prod over ray r of (1-alpha)

Strategy
--------
Shard by ray: each of the 8 cores gets a contiguous chunk of points that
covers a contiguous range of rays (chunk boundaries snapped to ray starts,
host-side searchsorted).  Inside a core the chunk is laid out as a
[128, L] grid; each partition row also starts at a ray boundary (host
marshals rows into the grid, padding short rows; row boundaries are
balanced by binary search on the max row length so L is minimal).
Because every row starts at a ray start, the per-ray segmented scan never
crosses a partition boundary and a single tensor_tensor_scan pass per
column tile computes it:   state = m * state + sp   (m = 0 at ray starts).
Column tiles follow a tapered width schedule (small first/last tiles)
to shorten the pipeline ramp and tail; loads issue from the SP HWDGE
queue, stores from the GPSIMD SWDGE queue so store data-waits never
block loads, and all activations resolve to one HW table set.

The scan is linear in its data input, so the -interval factor is folded
into the Exp activations' scale.  softplus is computed as ln(1+exp(x))
(Softplus has no HW activation table; Exp and Ln share one set).

Outputs per core: weights (per point) and expincl = exp(-interval *
inclusive-scan) per point; the host gathers alphainv_last[r] =
expincl[last point of ray r] (empty rays -> 1.0).
"""

import functools

import numpy as np

P = 128           # SBUF partitions
N_CORES = 8


def _tile_schedule(total_cols: int):
    """Tapered column-tile widths summing to total_cols (a multiple of 32).

    Small leading/trailing tiles shorten the pipeline ramp and tail; the
    fractions reproduce the best TimelineSim schedule at full size.
    """
    u = total_cols // 32
    cuts = [0, 9, 27, 62, 97, 132, 167, 202, 237, 259]
    starts = [round(c * u / 259) for c in cuts]
    tiles = [32 * (b - a) for a, b in zip(starts, starts[1:]) if b > a]
    assert sum(tiles) == total_cols
    return tuple(tiles)


@functools.lru_cache(maxsize=4)
def _get_program(tiles: tuple, shift: float, interval: float, rid16: bool):
    import concourse.bacc as bacc
    import concourse.mybir as mybir
    import concourse.tile as tile

    AF = mybir.ActivationFunctionType
    OP = mybir.AluOpType
    f32 = mybir.dt.float32
    bf16 = mybir.dt.bfloat16
    ridt = mybir.dt.int16 if rid16 else mybir.dt.int32
    ntiles = len(tiles)
    starts = [sum(tiles[:i]) for i in range(ntiles)]
    ftmax = max(tiles)
    L = sum(tiles)

    nc = bacc.Bacc("TRN2", target_bir_lowering=False, debug=False)
    dens = nc.dram_tensor("density_in", [P, L], f32, kind="ExternalInput")
    rid = nc.dram_tensor("rid_in", [P, L], ridt, kind="ExternalInput")
    wout = nc.dram_tensor("weights_out", [P, L], f32, kind="ExternalOutput")
    eout = nc.dram_tensor("expincl_out", [P, L], f32, kind="ExternalOutput")

    with tile.TileContext(nc) as tc:
        with (
            tc.tile_pool(name="main", bufs=3) as pool,
            tc.tile_pool(name="load", bufs=3) as lpool,
            tc.tile_pool(name="chain", bufs=2) as chain_pool,
            tc.tile_pool(name="small", bufs=1) as spool,
        ):
            shift_sb = spool.tile([P, 1], f32)
            nc.vector.memset(shift_sb[:], float(shift))

            incl_prev = None
            prev_ft = None
            for t in range(ntiles):
                ft = tiles[t]
                s0 = starts[t]
                cs = slice(s0, s0 + ft)
                # loads on SP (HWDGE): no data deps, never block the queue
                d_full = lpool.tile([P, ftmax], f32, tag="d")
                d_t = d_full[:, :ft]
                nc.sync.dma_start(out=d_t[:], in_=dens[:, cs])
                # rid tile loaded with one leading overlap column so the
                # whole continue-mask is one shifted compare (no cross-tile
                # chaining).  Tile 0's column -1 is a -1 sentinel; its mask
                # value is irrelevant because the scan's initial state is 0.
                r_full = lpool.tile([P, ftmax + 1], ridt, tag="r")
                r_t = r_full[:, :ft + 1]
                if t == 0:
                    nc.vector.memset(r_t[:, 0:1], -1)
                    nc.sync.dma_start(out=r_t[:, 1:], in_=rid[:, 0:ft])
                else:
                    nc.sync.dma_start(out=r_t[:], in_=rid[:, s0 - 1:s0 + ft])

                # softplus(x + shift) = ln(1 + exp(x + shift))
                # Exp in place on d (ACT -> ACT, same engine), then Ln to sp
                nc.scalar.activation(out=d_t[:], in_=d_t[:], func=AF.Exp,
                                     bias=shift_sb[:, :1], scale=1.0)
                sp_full = pool.tile([P, ftmax], f32, tag="sp")
                sp_t = sp_full[:, :ft]
                nc.scalar.activation(out=sp_t[:], in_=d_t[:], func=AF.Ln,
                                     bias=1.0, scale=1.0)

                # continue-mask: 1 where same ray as previous point, 0 at ray
                # starts (row starts are ray starts by construction).
                # bf16 mask (0/1 exact) lets the compare run in a faster DVE
                # mode; the scan upconverts operands to fp32 state.
                m_full = pool.tile([P, ftmax], bf16, tag="m")
                m_t = m_full[:, :ft]
                nc.vector.tensor_tensor(out=m_t[:], in0=r_t[:, 1:],
                                        in1=r_t[:, :ft], op=OP.is_equal)

                # segmented inclusive scan: state = m*state + sp
                incl_full = chain_pool.tile([P, ftmax], f32, tag="incl")
                incl_t = incl_full[:, :ft]
                nc.vector.tensor_tensor_scan(
                    out=incl_t[:], data0=m_t[:], data1=sp_t[:],
                    initial=(0.0 if t == 0 else
                             incl_prev[:, prev_ft - 1:prev_ft]),
                    op0=OP.mult, op1=OP.add,
                )

                # excl = incl - sp  (exclusive scan)
                excl_full = pool.tile([P, ftmax], f32, tag="excl")
                excl_t = excl_full[:, :ft]
                nc.vector.tensor_sub(out=excl_t[:], in0=incl_t[:], in1=sp_t[:])
                # T = exp(-interval * excl)
                T_full = pool.tile([P, ftmax], f32, tag="T")
                T_t = T_full[:, :ft]
                nc.scalar.activation(out=T_t[:], in_=excl_t[:], func=AF.Exp,
                                     scale=-float(interval))
                # expincl = exp(-interval * incl)
                ei_full = pool.tile([P, ftmax], f32, tag="ei")
                ei_t = ei_full[:, :ft]
                nc.scalar.activation(out=ei_t[:], in_=incl_t[:], func=AF.Exp,
                                     scale=-float(interval))
                # weights = T - expincl = alpha * T
                w_full = pool.tile([P, ftmax], f32, tag="w")
                w_t = w_full[:, :ft]
                nc.vector.tensor_sub(out=w_t[:], in0=T_t[:], in1=ei_t[:])

                # stores on Pool (SWDGE): their data-waits don't block loads
                nc.gpsimd.dma_start(out=wout[:, cs], in_=w_t[:])
                nc.gpsimd.dma_start(out=eout[:, cs], in_=ei_t[:])
                incl_prev, prev_ft = incl_t, ft

    # Compile with only activation tables containing BOTH Exp and Ln
    # offered, so every activation resolves to one table set (otherwise the
    # chooser alternates sets per tile, costing an ~2.7us table DMA each
    # time).  Scoped + restored: only affects this compile.
    AFt = mybir.ActivationFunctionType
    orig_tables = bacc.get_activation_tables

    def one_table(arch, _orig=orig_tables, _AF=AFt):
        tabs = _orig(arch)
        if not any(_AF.Exp in s and _AF.Ln in s for s in tabs.values()):
            return tabs
        # Keep every entry at its original position (the pass encodes the
        # table id positionally); empty the sets we don't want chosen.
        return {n: (s if (_AF.Exp in s and _AF.Ln in s) else set())
                for n, s in tabs.items()}

    bacc.get_activation_tables = one_table
    try:
        nc.compile()
    finally:
        bacc.get_activation_tables = orig_tables
    return nc


def _greedy_rows(ray_start_pts: np.ndarray, b0: int, b1: int, cap: int):
    """128 row starts (ray-aligned), each row <= cap points; None if infeasible.

    ray_start_pts: point index of each ray start in the chunk, with b1
    appended as a sentinel.  Greedy: each row extends to the furthest ray
    start within cap.
    """
    s = np.empty(P, np.int64)
    cur = b0
    for p in range(P):
        s[p] = cur
        if p == P - 1 or cur >= b1:
            continue
        j = np.searchsorted(ray_start_pts, cur + cap, side="right") - 1
        nxt = int(ray_start_pts[j])
        if nxt <= cur:
            return None  # one ray alone exceeds cap
        cur = min(nxt, b1)
    if b1 - int(s[P - 1]) > cap:
        return None
    return s


def _plan(rid: np.ndarray, n_rays: int, n_pts: int):
    """Host-side sharding: chunk + row boundaries, all snapped to ray starts.

    Row boundaries are balanced (binary search on the max row length) so
    the fixed row window L — and with it the padded DMA traffic — is
    minimal.
    """
    marks = (np.arange(1, N_CORES) * n_rays) // N_CORES
    bounds = np.concatenate(
        [[0], np.searchsorted(rid, marks, side="left"), [n_pts]]
    ).astype(np.int64)

    ray_marks = np.concatenate([[0], marks, [n_rays]])
    core_ray_pts = []
    max_len = 0
    for k in range(N_CORES):
        b0, b1 = int(bounds[k]), int(bounds[k + 1])
        r0, r1 = int(ray_marks[k]), int(ray_marks[k + 1])
        rsp = np.concatenate([
            np.searchsorted(rid, np.arange(r0, r1), side="left"), [b1]
        ]).astype(np.int64)
        core_ray_pts.append((rsp, b0, b1))
        lo, hi = -(-(b1 - b0) // P), b1 - b0
        while lo < hi:
            mid = (lo + hi) // 2
            if _greedy_rows(rsp, b0, b1, mid) is None:
                lo = mid + 1
            else:
                hi = mid
        max_len = max(max_len, lo)

    row_starts = []
    for (rsp, b0, b1) in core_ray_pts:
        s = _greedy_rows(rsp, b0, b1, max_len)
        assert s is not None
        row_starts.append(s)
    return bounds, row_starts, max_len


def _grid_shapes(rid, n_rays, n_pts):
    bounds, row_starts, max_len = _plan(rid, n_rays, n_pts)
    total = -(-max_len // 32) * 32
    tiles = _tile_schedule(total)
    return bounds, row_starts, tiles


def _make_in_maps(density, rid, bounds, row_starts, tiles, rid16):
    L = sum(tiles)
    in_maps = []
    for k in range(N_CORES):
        b0, b1 = int(bounds[k]), int(bounds[k + 1])
        s = row_starts[k]
        lens = np.diff(np.append(s, b1))
        d_grid = np.zeros((P, L), np.float32)
        r_grid = np.zeros((P, L), np.int16 if rid16 else np.int32)
        for p in range(P):
            n = int(lens[p])
            if n:
                d_grid[p, :n] = density[s[p]:s[p] + n]
                seg = rid[s[p]:s[p] + n]
                if rid16:
                    seg = seg.astype(np.uint16).view(np.int16)
                r_grid[p, :n] = seg
        in_maps.append({"density_in": d_grid, "rid_in": r_grid})
    return in_maps


def kernel(density, shift, interval, ray_id, n_rays):
    from concourse.bass_utils import run_bass_kernel_spmd

    density = np.ascontiguousarray(np.asarray(density), dtype=np.float32).ravel()
    rid = np.ascontiguousarray(np.asarray(ray_id), dtype=np.int32).ravel()
    shift_f = float(np.asarray(shift))
    interval_f = float(np.asarray(interval))
    n_rays_i = int(np.asarray(n_rays))
    n_pts = density.shape[0]
    assert rid.shape[0] == n_pts

    bounds, row_starts, tiles = _grid_shapes(rid, n_rays_i, n_pts)
    rid16 = n_rays_i <= 65536
    nc = _get_program(tiles, shift_f, interval_f, rid16)
    in_maps = _make_in_maps(density, rid, bounds, row_starts, tiles, rid16)

    # One retry: transient NRT device wedges (NRT_EXEC_UNIT_UNRECOVERABLE)
    # have been observed to clear on re-execution.
    try:
        res = run_bass_kernel_spmd(nc, in_maps, list(range(N_CORES))).results
    except Exception:
        import time as _time
        _time.sleep(3.0)
        res = run_bass_kernel_spmd(nc, in_maps, list(range(N_CORES))).results

    weights = np.empty(n_pts, np.float32)
    expincl = np.empty(n_pts, np.float32)
    for k in range(N_CORES):
        b0, b1 = int(bounds[k]), int(bounds[k + 1])
        s = row_starts[k]
        lens = np.diff(np.append(s, b1))
        w_grid = res[k]["weights_out"]
        e_grid = res[k]["expincl_out"]
        for p in range(P):
            n = int(lens[p])
            if n:
                weights[s[p]:s[p] + n] = w_grid[p, :n]
                expincl[s[p]:s[p] + n] = e_grid[p, :n]

    ray_ids = np.arange(n_rays_i)
    ends = np.searchsorted(rid, ray_ids, side="right") - 1
    starts = np.searchsorted(rid, ray_ids, side="left")
    alphainv_last = np.ones(n_rays_i, np.float32)
    nonempty = ends >= starts
    alphainv_last[nonempty] = expincl[ends[nonempty]]
    return weights, alphainv_last


# revision 26
# speedup vs baseline: 1.0020x; 1.0020x over previous
"""DirectVoxGO Raw2Alpha + Alphas2Weights (segmented scan) on 8 Trainium2 cores.

Computes, for n_pts sample points sorted by ray_id:
    sp      = softplus(density + shift)
    log(1-alpha) = -interval * sp
    weights = alpha * T   (T = exclusive per-ray cumprod of (1-alpha))
    alphainv_last[r] = prod over ray r of (1-alpha)

Strategy
--------
Shard by ray: each of the 8 cores gets a contiguous chunk of points that
covers a contiguous range of rays (chunk boundaries snapped to ray starts,
host-side searchsorted).  Inside a core the chunk is laid out as a
[128, L] grid; each partition row also starts at a ray boundary (host
marshals rows into the grid, padding short rows; row boundaries are
balanced by binary search on the max row length so L is minimal).
Because every row starts at a ray start, the per-ray segmented scan never
crosses a partition boundary and a single tensor_tensor_scan pass per
column tile computes it:   state = m * state + sp   (m = 0 at ray starts).
Column tiles follow a tapered width schedule (small first/last tiles)
to shorten the pipeline ramp and tail; loads issue from the SP HWDGE
queue, stores from the GPSIMD SWDGE queue so store data-waits never
block loads, and all activations resolve to one HW table set.

The scan is linear in its data input, so the -interval factor is folded
into the Exp activations' scale.  softplus is computed as ln(1+exp(x))
(Softplus has no HW activation table; Exp and Ln share one set).

Outputs per core: weights (per point) and expincl = exp(-interval *
inclusive-scan) per point; the host gathers alphainv_last[r] =
expincl[last point of ray r] (empty rays -> 1.0).
"""

import functools

import numpy as np

P = 128           # SBUF partitions
N_CORES = 8


def _tile_schedule(total_cols: int):
    """Tapered column-tile widths summing to total_cols (a multiple of 8).

    Small leading/trailing tiles shorten the pipeline ramp and tail; the
    fractions reproduce the best TimelineSim schedule at full size.
    """
    u = total_cols // 8
    cuts = [0, 9, 27, 62, 97, 132, 167, 202, 237, 259]
    starts = [round(c * u / 259) for c in cuts]
    tiles = [8 * (b - a) for a, b in zip(starts, starts[1:]) if b > a]
    assert sum(tiles) == total_cols
    return tuple(tiles)


@functools.lru_cache(maxsize=4)
def _get_program(tiles: tuple, shift: float, interval: float, rid16: bool):
    import concourse.bacc as bacc
    import concourse.mybir as mybir
    import concourse.tile as tile

    AF = mybir.ActivationFunctionType
    OP = mybir.AluOpType
    f32 = mybir.dt.float32
    bf16 = mybir.dt.bfloat16
    ridt = mybir.dt.int16 if rid16 else mybir.dt.int32
    ntiles = len(tiles)
    starts = [sum(tiles[:i]) for i in range(ntiles)]
    ftmax = max(tiles)
    L = sum(tiles)

    nc = bacc.Bacc("TRN2", target_bir_lowering=False, debug=False)
    dens = nc.dram_tensor("density_in", [P, L], f32, kind="ExternalInput")
    rid = nc.dram_tensor("rid_in", [P, L], ridt, kind="ExternalInput")
    wout = nc.dram_tensor("weights_out", [P, L], f32, kind="ExternalOutput")
    eout = nc.dram_tensor("expincl_out", [P, L], f32, kind="ExternalOutput")

    with tile.TileContext(nc) as tc:
        with (
            tc.tile_pool(name="main", bufs=3) as pool,
            tc.tile_pool(name="load", bufs=3) as lpool,
            tc.tile_pool(name="chain", bufs=2) as chain_pool,
            tc.tile_pool(name="small", bufs=1) as spool,
        ):
            shift_sb = spool.tile([P, 1], f32)
            nc.vector.memset(shift_sb[:], float(shift))

            incl_prev = None
            prev_ft = None
            for t in range(ntiles):
                ft = tiles[t]
                s0 = starts[t]
                cs = slice(s0, s0 + ft)
                # loads on SP (HWDGE): no data deps, never block the queue
                d_full = lpool.tile([P, ftmax], f32, tag="d")
                d_t = d_full[:, :ft]
                nc.sync.dma_start(out=d_t[:], in_=dens[:, cs])
                # rid tile loaded with one leading overlap column so the
                # whole continue-mask is one shifted compare (no cross-tile
                # chaining).  Tile 0's column -1 is a -1 sentinel; its mask
                # value is irrelevant because the scan's initial state is 0.
                r_full = lpool.tile([P, ftmax + 1], ridt, tag="r")
                r_t = r_full[:, :ft + 1]
                if t == 0:
                    nc.vector.memset(r_t[:, 0:1], -1)
                    nc.sync.dma_start(out=r_t[:, 1:], in_=rid[:, 0:ft])
                else:
                    nc.sync.dma_start(out=r_t[:], in_=rid[:, s0 - 1:s0 + ft])

                # softplus(x + shift) = ln(1 + exp(x + shift))
                # Exp in place on d (ACT -> ACT, same engine), then Ln to sp
                nc.scalar.activation(out=d_t[:], in_=d_t[:], func=AF.Exp,
                                     bias=shift_sb[:, :1], scale=1.0)
                sp_full = pool.tile([P, ftmax], f32, tag="sp")
                sp_t = sp_full[:, :ft]
                nc.scalar.activation(out=sp_t[:], in_=d_t[:], func=AF.Ln,
                                     bias=1.0, scale=1.0)

                # continue-mask: 1 where same ray as previous point, 0 at ray
                # starts (row starts are ray starts by construction).
                # bf16 mask (0/1 exact) lets the compare run in a faster DVE
                # mode; the scan upconverts operands to fp32 state.
                m_full = pool.tile([P, ftmax], bf16, tag="m")
                m_t = m_full[:, :ft]
                nc.vector.tensor_tensor(out=m_t[:], in0=r_t[:, 1:],
                                        in1=r_t[:, :ft], op=OP.is_equal)

                # segmented inclusive scan: state = m*state + sp
                incl_full = chain_pool.tile([P, ftmax], f32, tag="incl")
                incl_t = incl_full[:, :ft]
                nc.vector.tensor_tensor_scan(
                    out=incl_t[:], data0=m_t[:], data1=sp_t[:],
                    initial=(0.0 if t == 0 else
                             incl_prev[:, prev_ft - 1:prev_ft]),
                    op0=OP.mult, op1=OP.add,
                )

                # excl = incl - sp  (exclusive scan)
                excl_full = pool.tile([P, ftmax], f32, tag="excl")
                excl_t = excl_full[:, :ft]
                nc.vector.tensor_sub(out=excl_t[:], in0=incl_t[:], in1=sp_t[:])
                # T = exp(-interval * excl)
                T_full = pool.tile([P, ftmax], f32, tag="T")
                T_t = T_full[:, :ft]
                nc.scalar.activation(out=T_t[:], in_=excl_t[:], func=AF.Exp,
                                     scale=-float(interval))
                # expincl = exp(-interval * incl)
                ei_full = pool.tile([P, ftmax], f32, tag="ei")
                ei_t = ei_full[:, :ft]
                nc.scalar.activation(out=ei_t[:], in_=incl_t[:], func=AF.Exp,
                                     scale=-float(interval))
                # weights = T - expincl = alpha * T
                w_full = pool.tile([P, ftmax], f32, tag="w")
                w_t = w_full[:, :ft]
                nc.vector.tensor_sub(out=w_t[:], in0=T_t[:], in1=ei_t[:])

                # stores on Pool (SWDGE): their data-waits don't block loads
                nc.gpsimd.dma_start(out=wout[:, cs], in_=w_t[:])
                nc.gpsimd.dma_start(out=eout[:, cs], in_=ei_t[:])
                incl_prev, prev_ft = incl_t, ft

    # Compile with only activation tables containing BOTH Exp and Ln
    # offered, so every activation resolves to one table set (otherwise the
    # chooser alternates sets per tile, costing an ~2.7us table DMA each
    # time).  Scoped + restored: only affects this compile.
    AFt = mybir.ActivationFunctionType
    orig_tables = bacc.get_activation_tables

    def one_table(arch, _orig=orig_tables, _AF=AFt):
        tabs = _orig(arch)
        if not any(_AF.Exp in s and _AF.Ln in s for s in tabs.values()):
            return tabs
        # Keep every entry at its original position (the pass encodes the
        # table id positionally); empty the sets we don't want chosen.
        return {n: (s if (_AF.Exp in s and _AF.Ln in s) else set())
                for n, s in tabs.items()}

    bacc.get_activation_tables = one_table
    try:
        nc.compile()
    finally:
        bacc.get_activation_tables = orig_tables
    return nc


def _greedy_rows(ray_start_pts: np.ndarray, b0: int, b1: int, cap: int):
    """128 row starts (ray-aligned), each row <= cap points; None if infeasible.

    ray_start_pts: point index of each ray start in the chunk, with b1
    appended as a sentinel.  Greedy: each row extends to the furthest ray
    start within cap.
    """
    s = np.empty(P, np.int64)
    cur = b0
    for p in range(P):
        s[p] = cur
        if p == P - 1 or cur >= b1:
            continue
        j = np.searchsorted(ray_start_pts, cur + cap, side="right") - 1
        nxt = int(ray_start_pts[j])
        if nxt <= cur:
            return None  # one ray alone exceeds cap
        cur = min(nxt, b1)
    if b1 - int(s[P - 1]) > cap:
        return None
    return s


def _plan(rid: np.ndarray, n_rays: int, n_pts: int):
    """Host-side sharding: chunk + row boundaries, all snapped to ray starts.

    Row boundaries are balanced (binary search on the max row length) so
    the fixed row window L — and with it the padded DMA traffic — is
    minimal.
    """
    marks = (np.arange(1, N_CORES) * n_rays) // N_CORES
    bounds = np.concatenate(
        [[0], np.searchsorted(rid, marks, side="left"), [n_pts]]
    ).astype(np.int64)

    ray_marks = np.concatenate([[0], marks, [n_rays]])
    core_ray_pts = []
    max_len = 0
    for k in range(N_CORES):
        b0, b1 = int(bounds[k]), int(bounds[k + 1])
        r0, r1 = int(ray_marks[k]), int(ray_marks[k + 1])
        rsp = np.concatenate([
            np.searchsorted(rid, np.arange(r0, r1), side="left"), [b1]
        ]).astype(np.int64)
        core_ray_pts.append((rsp, b0, b1))
        lo, hi = -(-(b1 - b0) // P), b1 - b0
        while lo < hi:
            mid = (lo + hi) // 2
            if _greedy_rows(rsp, b0, b1, mid) is None:
                lo = mid + 1
            else:
                hi = mid
        max_len = max(max_len, lo)

    row_starts = []
    for (rsp, b0, b1) in core_ray_pts:
        s = _greedy_rows(rsp, b0, b1, max_len)
        assert s is not None
        row_starts.append(s)
    return bounds, row_starts, max_len


def _grid_shapes(rid, n_rays, n_pts):
    bounds, row_starts, max_len = _plan(rid, n_rays, n_pts)
    total = -(-max_len // 8) * 8
    tiles = _tile_schedule(total)
    return bounds, row_starts, tiles


def _make_in_maps(density, rid, bounds, row_starts, tiles, rid16):
    L = sum(tiles)
    in_maps = []
    for k in range(N_CORES):
        b0, b1 = int(bounds[k]), int(bounds[k + 1])
        s = row_starts[k]
        lens = np.diff(np.append(s, b1))
        d_grid = np.zeros((P, L), np.float32)
        r_grid = np.zeros((P, L), np.int16 if rid16 else np.int32)
        for p in range(P):
            n = int(lens[p])
            if n:
                d_grid[p, :n] = density[s[p]:s[p] + n]
                seg = rid[s[p]:s[p] + n]
                if rid16:
                    seg = seg.astype(np.uint16).view(np.int16)
                r_grid[p, :n] = seg
        in_maps.append({"density_in": d_grid, "rid_in": r_grid})
    return in_maps


def kernel(density, shift, interval, ray_id, n_rays):
    from concourse.bass_utils import run_bass_kernel_spmd

    density = np.ascontiguousarray(np.asarray(density), dtype=np.float32).ravel()
    rid = np.ascontiguousarray(np.asarray(ray_id), dtype=np.int32).ravel()
    shift_f = float(np.asarray(shift))
    interval_f = float(np.asarray(interval))
    n_rays_i = int(np.asarray(n_rays))
    n_pts = density.shape[0]
    assert rid.shape[0] == n_pts

    bounds, row_starts, tiles = _grid_shapes(rid, n_rays_i, n_pts)
    rid16 = n_rays_i <= 65536
    nc = _get_program(tiles, shift_f, interval_f, rid16)
    in_maps = _make_in_maps(density, rid, bounds, row_starts, tiles, rid16)

    # One retry: transient NRT device wedges (NRT_EXEC_UNIT_UNRECOVERABLE)
    # have been observed to clear on re-execution.
    try:
        res = run_bass_kernel_spmd(nc, in_maps, list(range(N_CORES))).results
    except Exception:
        import time as _time
        _time.sleep(3.0)
        res = run_bass_kernel_spmd(nc, in_maps, list(range(N_CORES))).results

    weights = np.empty(n_pts, np.float32)
    expincl = np.empty(n_pts, np.float32)
    for k in range(N_CORES):
        b0, b1 = int(bounds[k]), int(bounds[k + 1])
        s = row_starts[k]
        lens = np.diff(np.append(s, b1))
        w_grid = res[k]["weights_out"]
        e_grid = res[k]["expincl_out"]
        for p in range(P):
            n = int(lens[p])
            if n:
                weights[s[p]:s[p] + n] = w_grid[p, :n]
                expincl[s[p]:s[p] + n] = e_grid[p, :n]

    ray_ids = np.arange(n_rays_i)
    ends = np.searchsorted(rid, ray_ids, side="right") - 1
    starts = np.searchsorted(rid, ray_ids, side="left")
    alphainv_last = np.ones(n_rays_i, np.float32)
    nonempty = ends >= starts
    alphainv_last[nonempty] = expincl[ends[nonempty]]
    return weights, alphainv_last


# revision 28
# speedup vs baseline: 1.0046x; 1.0026x over previous
"""DirectVoxGO Raw2Alpha + Alphas2Weights (segmented scan) on 8 Trainium2 cores.

Computes, for n_pts sample points sorted by ray_id:
    sp      = softplus(density + shift)
    log(1-alpha) = -interval * sp
    weights = alpha * T   (T = exclusive per-ray cumprod of (1-alpha))
    alphainv_last[r] = prod over ray r of (1-alpha)

Strategy
--------
Shard by ray: each of the 8 cores gets a contiguous chunk of points that
covers a contiguous range of rays (chunk boundaries snapped to ray starts,
host-side searchsorted).  Inside a core the chunk is laid out as a
[128, L] grid; each partition row also starts at a ray boundary (host
marshals rows into the grid, padding short rows; row boundaries are
balanced by binary search on the max row length so L is minimal).
Because every row starts at a ray start, the per-ray segmented scan never
crosses a partition boundary and a single tensor_tensor_scan pass per
column tile computes it:   state = m * state + sp   (m = 0 at ray starts).
Column tiles follow a tapered width schedule (small first/last tiles)
to shorten the pipeline ramp and tail; loads issue from the SP HWDGE
queue, stores from the GPSIMD SWDGE queue so store data-waits never
block loads, and all activations resolve to one HW table set.

The scan is linear in its data input, so the -interval factor is folded
into the Exp activations' scale.  softplus is computed as ln(1+exp(x))
(Softplus has no HW activation table; Exp and Ln share one set).

Outputs per core: weights (per point) and expincl = exp(-interval *
inclusive-scan) per point; the host gathers alphainv_last[r] =
expincl[last point of ray r] (empty rays -> 1.0).
"""

import functools

import numpy as np

P = 128           # SBUF partitions
N_CORES = 8


def _tile_schedule(total_cols: int):
    """Tapered column-tile widths summing to total_cols (a multiple of 8).

    Small leading/trailing tiles shorten the pipeline ramp and tail; the
    fractions reproduce the best TimelineSim schedule at full size.
    """
    u = total_cols // 8
    cuts = [0, 9, 27, 62, 97, 132, 167, 202, 237, 259]
    starts = [round(c * u / 259) for c in cuts]
    tiles = [8 * (b - a) for a, b in zip(starts, starts[1:]) if b > a]
    assert sum(tiles) == total_cols
    return tuple(tiles)


@functools.lru_cache(maxsize=4)
def _get_program(tiles: tuple, shift: float, interval: float, rid16: bool):
    import concourse.bacc as bacc
    import concourse.mybir as mybir
    import concourse.tile as tile

    AF = mybir.ActivationFunctionType
    OP = mybir.AluOpType
    f32 = mybir.dt.float32
    bf16 = mybir.dt.bfloat16
    ridt = mybir.dt.int16 if rid16 else mybir.dt.int32
    ntiles = len(tiles)
    starts = [sum(tiles[:i]) for i in range(ntiles)]
    ftmax = max(tiles)
    L = sum(tiles)

    nc = bacc.Bacc("TRN2", target_bir_lowering=False, debug=False)
    dens = nc.dram_tensor("density_in", [P, L], f32, kind="ExternalInput")
    rid = nc.dram_tensor("rid_in", [P, L], ridt, kind="ExternalInput")
    wout = nc.dram_tensor("weights_out", [P, L], f32, kind="ExternalOutput")
    eout = nc.dram_tensor("expincl_out", [P, L], f32, kind="ExternalOutput")

    with tile.TileContext(nc) as tc:
        with (
            tc.tile_pool(name="main", bufs=3) as pool,
            tc.tile_pool(name="load", bufs=3) as lpool,
            tc.tile_pool(name="chain", bufs=2) as chain_pool,
            tc.tile_pool(name="small", bufs=1) as spool,
        ):
            shift_sb = spool.tile([P, 1], f32)
            nc.vector.memset(shift_sb[:], float(shift))

            incl_prev = None
            prev_ft = None
            for t in range(ntiles):
                ft = tiles[t]
                s0 = starts[t]
                cs = slice(s0, s0 + ft)
                # loads on SP (HWDGE): no data deps, never block the queue.
                # Tile 0's loads go on the ACT ring instead — it is idle
                # until the first load lands, so the two rings fill the
                # startup ramp in parallel.
                load_eng = nc.scalar if t == 0 else nc.sync
                d_full = lpool.tile([P, ftmax], f32, tag="d")
                d_t = d_full[:, :ft]
                load_eng.dma_start(out=d_t[:], in_=dens[:, cs])
                # rid tile loaded with one leading overlap column so the
                # whole continue-mask is one shifted compare (no cross-tile
                # chaining).  Tile 0's column -1 is a -1 sentinel; its mask
                # value is irrelevant because the scan's initial state is 0.
                r_full = lpool.tile([P, ftmax + 1], ridt, tag="r")
                r_t = r_full[:, :ft + 1]
                if t == 0:
                    nc.vector.memset(r_t[:, 0:1], -1)
                    load_eng.dma_start(out=r_t[:, 1:], in_=rid[:, 0:ft])
                else:
                    load_eng.dma_start(out=r_t[:], in_=rid[:, s0 - 1:s0 + ft])

                # softplus(x + shift) = ln(1 + exp(x + shift))
                # Exp in place on d (ACT -> ACT, same engine), then Ln to sp
                nc.scalar.activation(out=d_t[:], in_=d_t[:], func=AF.Exp,
                                     bias=shift_sb[:, :1], scale=1.0)
                sp_full = pool.tile([P, ftmax], f32, tag="sp")
                sp_t = sp_full[:, :ft]
                nc.scalar.activation(out=sp_t[:], in_=d_t[:], func=AF.Ln,
                                     bias=1.0, scale=1.0)

                # continue-mask: 1 where same ray as previous point, 0 at ray
                # starts (row starts are ray starts by construction).
                # bf16 mask (0/1 exact) lets the compare run in a faster DVE
                # mode; the scan upconverts operands to fp32 state.
                m_full = pool.tile([P, ftmax], bf16, tag="m")
                m_t = m_full[:, :ft]
                nc.vector.tensor_tensor(out=m_t[:], in0=r_t[:, 1:],
                                        in1=r_t[:, :ft], op=OP.is_equal)

                # segmented inclusive scan: state = m*state + sp
                incl_full = chain_pool.tile([P, ftmax], f32, tag="incl")
                incl_t = incl_full[:, :ft]
                nc.vector.tensor_tensor_scan(
                    out=incl_t[:], data0=m_t[:], data1=sp_t[:],
                    initial=(0.0 if t == 0 else
                             incl_prev[:, prev_ft - 1:prev_ft]),
                    op0=OP.mult, op1=OP.add,
                )

                # excl = incl - sp  (exclusive scan)
                excl_full = pool.tile([P, ftmax], f32, tag="excl")
                excl_t = excl_full[:, :ft]
                nc.vector.tensor_sub(out=excl_t[:], in0=incl_t[:], in1=sp_t[:])
                # T = exp(-interval * excl)
                T_full = pool.tile([P, ftmax], f32, tag="T")
                T_t = T_full[:, :ft]
                nc.scalar.activation(out=T_t[:], in_=excl_t[:], func=AF.Exp,
                                     scale=-float(interval))
                # expincl = exp(-interval * incl)
                ei_full = pool.tile([P, ftmax], f32, tag="ei")
                ei_t = ei_full[:, :ft]
                nc.scalar.activation(out=ei_t[:], in_=incl_t[:], func=AF.Exp,
                                     scale=-float(interval))
                # weights = T - expincl = alpha * T
                w_full = pool.tile([P, ftmax], f32, tag="w")
                w_t = w_full[:, :ft]
                nc.vector.tensor_sub(out=w_t[:], in0=T_t[:], in1=ei_t[:])

                # stores on Pool (SWDGE): their data-waits don't block loads
                nc.gpsimd.dma_start(out=wout[:, cs], in_=w_t[:])
                nc.gpsimd.dma_start(out=eout[:, cs], in_=ei_t[:])
                incl_prev, prev_ft = incl_t, ft

    # Compile with only activation tables containing BOTH Exp and Ln
    # offered, so every activation resolves to one table set (otherwise the
    # chooser alternates sets per tile, costing an ~2.7us table DMA each
    # time).  Scoped + restored: only affects this compile.
    AFt = mybir.ActivationFunctionType
    orig_tables = bacc.get_activation_tables

    def one_table(arch, _orig=orig_tables, _AF=AFt):
        tabs = _orig(arch)
        if not any(_AF.Exp in s and _AF.Ln in s for s in tabs.values()):
            return tabs
        # Keep every entry at its original position (the pass encodes the
        # table id positionally); empty the sets we don't want chosen.
        return {n: (s if (_AF.Exp in s and _AF.Ln in s) else set())
                for n, s in tabs.items()}

    bacc.get_activation_tables = one_table
    try:
        nc.compile()
    finally:
        bacc.get_activation_tables = orig_tables
    return nc


def _greedy_rows(ray_start_pts: np.ndarray, b0: int, b1: int, cap: int):
    """128 row starts (ray-aligned), each row <= cap points; None if infeasible.

    ray_start_pts: point index of each ray start in the chunk, with b1
    appended as a sentinel.  Greedy: each row extends to the furthest ray
    start within cap.
    """
    s = np.empty(P, np.int64)
    cur = b0
    for p in range(P):
        s[p] = cur
        if p == P - 1 or cur >= b1:
            continue
        j = np.searchsorted(ray_start_pts, cur + cap, side="right") - 1
        nxt = int(ray_start_pts[j])
        if nxt <= cur:
            return None  # one ray alone exceeds cap
        cur = min(nxt, b1)
    if b1 - int(s[P - 1]) > cap:
        return None
    return s


def _plan(rid: np.ndarray, n_rays: int, n_pts: int):
    """Host-side sharding: chunk + row boundaries, all snapped to ray starts.

    Row boundaries are balanced (binary search on the max row length) so
    the fixed row window L — and with it the padded DMA traffic — is
    minimal.
    """
    marks = (np.arange(1, N_CORES) * n_rays) // N_CORES
    bounds = np.concatenate(
        [[0], np.searchsorted(rid, marks, side="left"), [n_pts]]
    ).astype(np.int64)

    ray_marks = np.concatenate([[0], marks, [n_rays]])
    core_ray_pts = []
    max_len = 0
    for k in range(N_CORES):
        b0, b1 = int(bounds[k]), int(bounds[k + 1])
        r0, r1 = int(ray_marks[k]), int(ray_marks[k + 1])
        rsp = np.concatenate([
            np.searchsorted(rid, np.arange(r0, r1), side="left"), [b1]
        ]).astype(np.int64)
        core_ray_pts.append((rsp, b0, b1))
        lo, hi = -(-(b1 - b0) // P), b1 - b0
        while lo < hi:
            mid = (lo + hi) // 2
            if _greedy_rows(rsp, b0, b1, mid) is None:
                lo = mid + 1
            else:
                hi = mid
        max_len = max(max_len, lo)

    row_starts = []
    for (rsp, b0, b1) in core_ray_pts:
        s = _greedy_rows(rsp, b0, b1, max_len)
        assert s is not None
        row_starts.append(s)
    return bounds, row_starts, max_len


def _grid_shapes(rid, n_rays, n_pts):
    bounds, row_starts, max_len = _plan(rid, n_rays, n_pts)
    total = -(-max_len // 8) * 8
    tiles = _tile_schedule(total)
    return bounds, row_starts, tiles


def _make_in_maps(density, rid, bounds, row_starts, tiles, rid16):
    L = sum(tiles)
    in_maps = []
    for k in range(N_CORES):
        b0, b1 = int(bounds[k]), int(bounds[k + 1])
        s = row_starts[k]
        lens = np.diff(np.append(s, b1))
        d_grid = np.zeros((P, L), np.float32)
        r_grid = np.zeros((P, L), np.int16 if rid16 else np.int32)
        for p in range(P):
            n = int(lens[p])
            if n:
                d_grid[p, :n] = density[s[p]:s[p] + n]
                seg = rid[s[p]:s[p] + n]
                if rid16:
                    seg = seg.astype(np.uint16).view(np.int16)
                r_grid[p, :n] = seg
        in_maps.append({"density_in": d_grid, "rid_in": r_grid})
    return in_maps


def kernel(density, shift, interval, ray_id, n_rays):
    from concourse.bass_utils import run_bass_kernel_spmd

    density = np.ascontiguousarray(np.asarray(density), dtype=np.float32).ravel()
    rid = np.ascontiguousarray(np.asarray(ray_id), dtype=np.int32).ravel()
    shift_f = float(np.asarray(shift))
    interval_f = float(np.asarray(interval))
    n_rays_i = int(np.asarray(n_rays))
    n_pts = density.shape[0]
    assert rid.shape[0] == n_pts

    bounds, row_starts, tiles = _grid_shapes(rid, n_rays_i, n_pts)
    rid16 = n_rays_i <= 65536
    nc = _get_program(tiles, shift_f, interval_f, rid16)
    in_maps = _make_in_maps(density, rid, bounds, row_starts, tiles, rid16)

    # One retry: transient NRT device wedges (NRT_EXEC_UNIT_UNRECOVERABLE)
    # have been observed to clear on re-execution.
    try:
        res = run_bass_kernel_spmd(nc, in_maps, list(range(N_CORES))).results
    except Exception:
        import time as _time
        _time.sleep(3.0)
        res = run_bass_kernel_spmd(nc, in_maps, list(range(N_CORES))).results

    weights = np.empty(n_pts, np.float32)
    expincl = np.empty(n_pts, np.float32)
    for k in range(N_CORES):
        b0, b1 = int(bounds[k]), int(bounds[k + 1])
        s = row_starts[k]
        lens = np.diff(np.append(s, b1))
        w_grid = res[k]["weights_out"]
        e_grid = res[k]["expincl_out"]
        for p in range(P):
            n = int(lens[p])
            if n:
                weights[s[p]:s[p] + n] = w_grid[p, :n]
                expincl[s[p]:s[p] + n] = e_grid[p, :n]

    ray_ids = np.arange(n_rays_i)
    ends = np.searchsorted(rid, ray_ids, side="right") - 1
    starts = np.searchsorted(rid, ray_ids, side="left")
    alphainv_last = np.ones(n_rays_i, np.float32)
    nonempty = ends >= starts
    alphainv_last[nonempty] = expincl[ends[nonempty]]
    return weights, alphainv_last


# revision 29
# speedup vs baseline: 1.0057x; 1.0011x over previous
"""DirectVoxGO Raw2Alpha + Alphas2Weights (segmented scan) on 8 Trainium2 cores.

Computes, for n_pts sample points sorted by ray_id:
    sp      = softplus(density + shift)
    log(1-alpha) = -interval * sp
    weights = alpha * T   (T = exclusive per-ray cumprod of (1-alpha))
    alphainv_last[r] = prod over ray r of (1-alpha)

Strategy
--------
Shard by ray: each of the 8 cores gets a contiguous chunk of points that
covers a contiguous range of rays (chunk boundaries snapped to ray starts,
host-side searchsorted).  Inside a core the chunk is laid out as a
[128, L] grid; each partition row also starts at a ray boundary (host
marshals rows into the grid, padding short rows; row boundaries are
balanced by binary search on the max row length so L is minimal).
Because every row starts at a ray start, the per-ray segmented scan never
crosses a partition boundary and a single tensor_tensor_scan pass per
column tile computes it:   state = m * state + sp   (m = 0 at ray starts).
Column tiles follow a tapered width schedule (small first/last tiles)
to shorten the pipeline ramp and tail; loads issue from the SP HWDGE
queue, stores from the GPSIMD SWDGE queue so store data-waits never
block loads, and all activations resolve to one HW table set.

The scan is linear in its data input, so the -interval factor is folded
into the Exp activations' scale.  softplus is computed as ln(1+exp(x))
(Softplus has no HW activation table; Exp and Ln share one set).

Outputs per core: weights (per point) and expincl = exp(-interval *
inclusive-scan) per point; the host gathers alphainv_last[r] =
expincl[last point of ray r] (empty rays -> 1.0).
"""

import functools

import numpy as np

P = 128           # SBUF partitions
N_CORES = 8


def _tile_schedule(total_cols: int):
    """Tapered column-tile widths summing to total_cols (a multiple of 8).

    Small leading/trailing tiles shorten the pipeline ramp and tail; the
    fractions reproduce the best TimelineSim schedule at full size.
    """
    u = total_cols // 8
    cuts = [0, 9, 27, 62, 97, 132, 167, 202, 237, 259]
    starts = [round(c * u / 259) for c in cuts]
    tiles = [8 * (b - a) for a, b in zip(starts, starts[1:]) if b > a]
    assert sum(tiles) == total_cols
    return tuple(tiles)


@functools.lru_cache(maxsize=4)
def _get_program(tiles: tuple, shift: float, interval: float, rid16: bool):
    import concourse.bacc as bacc
    import concourse.mybir as mybir
    import concourse.tile as tile

    AF = mybir.ActivationFunctionType
    OP = mybir.AluOpType
    f32 = mybir.dt.float32
    bf16 = mybir.dt.bfloat16
    ridt = mybir.dt.int16 if rid16 else mybir.dt.int32
    ntiles = len(tiles)
    starts = [sum(tiles[:i]) for i in range(ntiles)]
    ftmax = max(tiles)
    L = sum(tiles)

    nc = bacc.Bacc("TRN2", target_bir_lowering=False, debug=False)
    dens = nc.dram_tensor("density_in", [P, L], f32, kind="ExternalInput")
    rid = nc.dram_tensor("rid_in", [P, L], ridt, kind="ExternalInput")
    wout = nc.dram_tensor("weights_out", [P, L], f32, kind="ExternalOutput")
    eout = nc.dram_tensor("expincl_out", [P, L], f32, kind="ExternalOutput")

    with tile.TileContext(nc) as tc:
        with (
            tc.tile_pool(name="main", bufs=3) as pool,
            tc.tile_pool(name="load", bufs=3) as lpool,
            tc.tile_pool(name="chain", bufs=2) as chain_pool,
            tc.tile_pool(name="small", bufs=1) as spool,
        ):
            shift_sb = spool.tile([P, 1], f32)
            nc.vector.memset(shift_sb[:], float(shift))

            incl_prev = None
            prev_ft = None
            for t in range(ntiles):
                ft = tiles[t]
                s0 = starts[t]
                cs = slice(s0, s0 + ft)
                # loads on SP (HWDGE): no data deps, never block the queue.
                # Tile 0's loads go on the ACT ring instead — it is idle
                # until the first load lands, so the two rings fill the
                # startup ramp in parallel.
                load_eng = nc.scalar if t == 0 else nc.sync
                d_full = lpool.tile([P, ftmax], f32, tag="d")
                d_t = d_full[:, :ft]
                load_eng.dma_start(out=d_t[:], in_=dens[:, cs])
                # rid tile loaded with one leading overlap column so the
                # whole continue-mask is one shifted compare (no cross-tile
                # chaining).  Tile 0's column -1 is a -1 sentinel; its mask
                # value is irrelevant because the scan's initial state is 0.
                r_full = lpool.tile([P, ftmax + 1], ridt, tag="r")
                r_t = r_full[:, :ft + 1]
                if t == 0:
                    nc.vector.memset(r_t[:, 0:1], -1)
                    load_eng.dma_start(out=r_t[:, 1:], in_=rid[:, 0:ft])
                else:
                    load_eng.dma_start(out=r_t[:], in_=rid[:, s0 - 1:s0 + ft])

                # softplus(x + shift) = ln(1 + exp(x + shift))
                # Exp in place on d (ACT -> ACT, same engine), then Ln to sp
                nc.scalar.activation(out=d_t[:], in_=d_t[:], func=AF.Exp,
                                     bias=shift_sb[:, :1], scale=1.0)
                sp_full = pool.tile([P, ftmax], f32, tag="sp")
                sp_t = sp_full[:, :ft]
                nc.scalar.activation(out=sp_t[:], in_=d_t[:], func=AF.Ln,
                                     bias=1.0, scale=1.0)

                # continue-mask: 1 where same ray as previous point, 0 at ray
                # starts (row starts are ray starts by construction).
                # bf16 mask (0/1 exact) lets the compare run in a faster DVE
                # mode; the scan upconverts operands to fp32 state.
                m_full = pool.tile([P, ftmax], bf16, tag="m")
                m_t = m_full[:, :ft]
                nc.vector.tensor_tensor(out=m_t[:], in0=r_t[:, 1:],
                                        in1=r_t[:, :ft], op=OP.is_equal)

                # segmented inclusive scan: state = m*state + sp
                incl_full = chain_pool.tile([P, ftmax], f32, tag="incl")
                incl_t = incl_full[:, :ft]
                nc.vector.tensor_tensor_scan(
                    out=incl_t[:], data0=m_t[:], data1=sp_t[:],
                    initial=(0.0 if t == 0 else
                             incl_prev[:, prev_ft - 1:prev_ft]),
                    op0=OP.mult, op1=OP.add,
                )

                # excl = incl - sp  (exclusive scan)
                excl_full = pool.tile([P, ftmax], f32, tag="excl")
                excl_t = excl_full[:, :ft]
                nc.vector.tensor_sub(out=excl_t[:], in0=incl_t[:], in1=sp_t[:])
                # T = exp(-interval * excl)
                T_full = pool.tile([P, ftmax], f32, tag="T")
                T_t = T_full[:, :ft]
                nc.scalar.activation(out=T_t[:], in_=excl_t[:], func=AF.Exp,
                                     scale=-float(interval))
                # expincl = exp(-interval * incl)
                ei_full = pool.tile([P, ftmax], f32, tag="ei")
                ei_t = ei_full[:, :ft]
                nc.scalar.activation(out=ei_t[:], in_=incl_t[:], func=AF.Exp,
                                     scale=-float(interval))
                # weights = T - expincl = alpha * T
                w_full = pool.tile([P, ftmax], f32, tag="w")
                w_t = w_full[:, :ft]
                nc.vector.tensor_sub(out=w_t[:], in0=T_t[:], in1=ei_t[:])

                # stores on Pool (SWDGE): their data-waits don't block loads.
                # The very last ei store rides the (by then idle) SP ring so
                # the final two stores drain in parallel.
                nc.gpsimd.dma_start(out=wout[:, cs], in_=w_t[:])
                (nc.sync if t == ntiles - 1 else nc.gpsimd).dma_start(
                    out=eout[:, cs], in_=ei_t[:])
                incl_prev, prev_ft = incl_t, ft

    # Compile with only activation tables containing BOTH Exp and Ln
    # offered, so every activation resolves to one table set (otherwise the
    # chooser alternates sets per tile, costing an ~2.7us table DMA each
    # time).  Scoped + restored: only affects this compile.
    AFt = mybir.ActivationFunctionType
    orig_tables = bacc.get_activation_tables

    def one_table(arch, _orig=orig_tables, _AF=AFt):
        tabs = _orig(arch)
        if not any(_AF.Exp in s and _AF.Ln in s for s in tabs.values()):
            return tabs
        # Keep every entry at its original position (the pass encodes the
        # table id positionally); empty the sets we don't want chosen.
        return {n: (s if (_AF.Exp in s and _AF.Ln in s) else set())
                for n, s in tabs.items()}

    bacc.get_activation_tables = one_table
    try:
        nc.compile()
    finally:
        bacc.get_activation_tables = orig_tables
    return nc


def _greedy_rows(ray_start_pts: np.ndarray, b0: int, b1: int, cap: int):
    """128 row starts (ray-aligned), each row <= cap points; None if infeasible.

    ray_start_pts: point index of each ray start in the chunk, with b1
    appended as a sentinel.  Greedy: each row extends to the furthest ray
    start within cap.
    """
    s = np.empty(P, np.int64)
    cur = b0
    for p in range(P):
        s[p] = cur
        if p == P - 1 or cur >= b1:
            continue
        j = np.searchsorted(ray_start_pts, cur + cap, side="right") - 1
        nxt = int(ray_start_pts[j])
        if nxt <= cur:
            return None  # one ray alone exceeds cap
        cur = min(nxt, b1)
    if b1 - int(s[P - 1]) > cap:
        return None
    return s


def _plan(rid: np.ndarray, n_rays: int, n_pts: int):
    """Host-side sharding: chunk + row boundaries, all snapped to ray starts.

    Row boundaries are balanced (binary search on the max row length) so
    the fixed row window L — and with it the padded DMA traffic — is
    minimal.
    """
    marks = (np.arange(1, N_CORES) * n_rays) // N_CORES
    bounds = np.concatenate(
        [[0], np.searchsorted(rid, marks, side="left"), [n_pts]]
    ).astype(np.int64)

    ray_marks = np.concatenate([[0], marks, [n_rays]])
    core_ray_pts = []
    max_len = 0
    for k in range(N_CORES):
        b0, b1 = int(bounds[k]), int(bounds[k + 1])
        r0, r1 = int(ray_marks[k]), int(ray_marks[k + 1])
        rsp = np.concatenate([
            np.searchsorted(rid, np.arange(r0, r1), side="left"), [b1]
        ]).astype(np.int64)
        core_ray_pts.append((rsp, b0, b1))
        lo, hi = -(-(b1 - b0) // P), b1 - b0
        while lo < hi:
            mid = (lo + hi) // 2
            if _greedy_rows(rsp, b0, b1, mid) is None:
                lo = mid + 1
            else:
                hi = mid
        max_len = max(max_len, lo)

    row_starts = []
    for (rsp, b0, b1) in core_ray_pts:
        s = _greedy_rows(rsp, b0, b1, max_len)
        assert s is not None
        row_starts.append(s)
    return bounds, row_starts, max_len


def _grid_shapes(rid, n_rays, n_pts):
    bounds, row_starts, max_len = _plan(rid, n_rays, n_pts)
    total = -(-max_len // 8) * 8
    tiles = _tile_schedule(total)
    return bounds, row_starts, tiles


def _make_in_maps(density, rid, bounds, row_starts, tiles, rid16):
    L = sum(tiles)
    in_maps = []
    for k in range(N_CORES):
        b0, b1 = int(bounds[k]), int(bounds[k + 1])
        s = row_starts[k]
        lens = np.diff(np.append(s, b1))
        d_grid = np.zeros((P, L), np.float32)
        r_grid = np.zeros((P, L), np.int16 if rid16 else np.int32)
        for p in range(P):
            n = int(lens[p])
            if n:
                d_grid[p, :n] = density[s[p]:s[p] + n]
                seg = rid[s[p]:s[p] + n]
                if rid16:
                    seg = seg.astype(np.uint16).view(np.int16)
                r_grid[p, :n] = seg
        in_maps.append({"density_in": d_grid, "rid_in": r_grid})
    return in_maps


def kernel(density, shift, interval, ray_id, n_rays):
    from concourse.bass_utils import run_bass_kernel_spmd

    density = np.ascontiguousarray(np.asarray(density), dtype=np.float32).ravel()
    rid = np.ascontiguousarray(np.asarray(ray_id), dtype=np.int32).ravel()
    shift_f = float(np.asarray(shift))
    interval_f = float(np.asarray(interval))
    n_rays_i = int(np.asarray(n_rays))
    n_pts = density.shape[0]
    assert rid.shape[0] == n_pts

    bounds, row_starts, tiles = _grid_shapes(rid, n_rays_i, n_pts)
    rid16 = n_rays_i <= 65536
    nc = _get_program(tiles, shift_f, interval_f, rid16)
    in_maps = _make_in_maps(density, rid, bounds, row_starts, tiles, rid16)

    # One retry: transient NRT device wedges (NRT_EXEC_UNIT_UNRECOVERABLE)
    # have been observed to clear on re-execution.
    try:
        res = run_bass_kernel_spmd(nc, in_maps, list(range(N_CORES))).results
    except Exception:
        import time as _time
        _time.sleep(3.0)
        res = run_bass_kernel_spmd(nc, in_maps, list(range(N_CORES))).results

    weights = np.empty(n_pts, np.float32)
    expincl = np.empty(n_pts, np.float32)
    for k in range(N_CORES):
        b0, b1 = int(bounds[k]), int(bounds[k + 1])
        s = row_starts[k]
        lens = np.diff(np.append(s, b1))
        w_grid = res[k]["weights_out"]
        e_grid = res[k]["expincl_out"]
        for p in range(P):
            n = int(lens[p])
            if n:
                weights[s[p]:s[p] + n] = w_grid[p, :n]
                expincl[s[p]:s[p] + n] = e_grid[p, :n]

    ray_ids = np.arange(n_rays_i)
    ends = np.searchsorted(rid, ray_ids, side="right") - 1
    starts = np.searchsorted(rid, ray_ids, side="left")
    alphainv_last = np.ones(n_rays_i, np.float32)
    nonempty = ends >= starts
    alphainv_last[nonempty] = expincl[ends[nonempty]]
    return weights, alphainv_last
